# revision 3
# baseline (speedup 1.0000x reference)
"""GCNConv Trainium2 Bass kernel v3 (8 NeuronCores, SPMD).

vs v2.1:
  - psum->SBUF copies on the scalar (ACT) engine, freeing DVE for window adds
  - optional 2-way column-split AllGather (CS=2): two half-width collectives
    per rep (parallel collective queues), windows split per half, two
    ExternalOutputs reassembled on host
Engine/queue plan: gpsimd = phase-A DMAs + AG triggers only; sync ring =
A-half windows (+outA store); scalar ring = B-half windows + pads; DVE =
window adds (f16); ACT = psum copies; PE = matmuls.
"""

from contextlib import ExitStack

import numpy as np

import concourse.bacc as bacc
import concourse.bass as bass
import concourse.mybir as mybir
import concourse.tile as tile


def build_gcn_nc(
    N: int,
    DIN: int,
    D: int,
    DEG: int,
    M: int,
    reps: int = 1,
    phases: str = "ABW",
    CS: int = 2,      # column split of the AllGather (1 or 2)
    PAD2: bool = False,  # pad region via a small early AllGather instead of a local copy
):
    assert N % M == 0
    SH = N // M
    RPB = (SH + 127) // 128
    SH_PAD = 128 * RPB
    FREE = RPB * D
    NT = SH_PAD // 128
    KC = (DIN + 127) // 128
    assert DIN % KC == 0
    KSZ = DIN // KC
    assert SH_PAD <= N
    assert D % CS == 0
    DC = D // CS          # cols per split
    FREEC = RPB * DC

    f32 = mybir.dt.float32
    i32 = mybir.dt.int32
    f16 = mybir.dt.float16

    PADB = SH_PAD // M            # pad rows computed per rank (8*PADB == SH_PAD)
    assert SH_PAD % M == 0
    XW = SH_PAD + (PADB if PAD2 else 0)   # xt columns incl. redundant pad rows

    nc = bacc.Bacc("TRN2", num_devices=M)

    xt = nc.dram_tensor("xt", [DIN, XW], f16, kind="ExternalInput")
    w = nc.dram_tensor("w", [DIN, D], f16, kind="ExternalInput")
    starts = nc.dram_tensor("starts", [DEG, 1], i32, kind="ExternalInput")
    outs = [
        nc.dram_tensor(f"out{c}", [SH_PAD, DC], f16, kind="ExternalOutput")
        for c in range(CS)
    ]

    # [rep parity][column split]
    h_locs = [[nc.dram_tensor(f"h_loc{i}_{c}", [SH * DC], f16)
               for c in range(CS)] for i in range(2)]
    h_fullps = [[nc.dram_tensor(f"h_fullp{i}_{c}", [(N + SH_PAD) * DC], f16,
                                addr_space="Shared")
                 for c in range(CS)] for i in range(2)]
    h_padlocs = [[nc.dram_tensor(f"h_padloc{i}_{c}", [PADB * DC], f16)
                  for c in range(CS)] for i in range(2)] if PAD2 else None

    with tile.TileContext(nc) as tc, ExitStack() as ctx:
        pw = ctx.enter_context(tc.tile_pool(name="pw", bufs=1))
        pxt = ctx.enter_context(tc.tile_pool(name="pxt", bufs=1))
        phg = ctx.enter_context(tc.tile_pool(name="phg", bufs=2))
        psum = ctx.enter_context(tc.tile_pool(name="psum", bufs=1, space="PSUM"))
        pacc = ctx.enter_context(tc.tile_pool(name="pacc", bufs=1))
        pwin = ctx.enter_context(tc.tile_pool(name="pwin", bufs=4))
        for _rep in range(reps):
            h_loc = h_locs[_rep % 2]
            h_fullp = h_fullps[_rep % 2]

            # ---------------- phase A ----------------
            w_sb = pw.tile([KSZ, KC * D], f16, tag="w")
            w_r = w.rearrange("(c p) d -> p c d", c=KC, p=KSZ)
            nc.gpsimd.dma_start(out=w_sb[:], in_=w_r[:, :, :])

            xt_sb = pxt.tile([KSZ, KC * XW], f16, tag="xts")
            xt_r = xt.rearrange("(c p) s -> p c s", c=KC, p=KSZ)
            CHW = 2048
            # pad columns first so the pad AllGather can fire early
            ranges = ([(SH_PAD, XW)] if PAD2 else []) + \
                [(lo, min(lo + CHW, SH_PAD)) for lo in range(0, SH_PAD, CHW)]
            for lo, hi in ranges:
                nc.gpsimd.dma_start(
                    out=bass.AP(xt_sb.tensor, xt_sb.offset + lo,
                                [[xt_sb.tensor.shape[-1], KSZ],
                                 [XW, KC], [1, hi - lo]]),
                    in_=xt_r[:, :, lo:hi],
                )

            SGMAX = (4096 - 512) // D
            BANK = 512 // D
            if NT <= SGMAX:
                bounds = [0, NT]
                MAIN = NT
            else:
                b0 = ((SGMAX - 1) // BANK) * BANK
                b0 = b0 + (SGMAX - b0) // 2
                b0 = min(b0, SGMAX - 2)
                MAIN = (min(b0, SGMAX) // BANK) * BANK
                assert NT - b0 <= MAIN - 2
                bounds = [0, b0, NT]

            def region_of(t):
                if t < bounds[1]:
                    return t
                return MAIN - 1 - (t - bounds[1])

            # aux psum bank: scr dummy (last 8 cols) + pad-batch regions share it
            aux = psum.tile([128, 512], f32, tag="aux")
            scr = aux
            nc.tensor.matmul(
                out=scr[0:1, 504:505], lhsT=w_sb[0:1, 0:1], rhs=w_sb[0:1, 0:1],
                start=True, stop=True,
            )

            # ---- redundant pad-slice compute + early tiny AllGather ----
            if PAD2:
                NPT = (PADB + 127) // 128          # pad strips (incl. partial)
                PB = 7                              # strips per psum batch
                for blo in range(0, NPT, PB):
                    bhi = min(blo + PB, NPT)
                    ppad = aux
                    for t in range(blo, bhi):
                        ncol = min(128, PADB - t * 128)
                        for c in range(KC):
                            nc.tensor.matmul(
                                out=ppad[:ncol, (t - blo) * D:(t - blo + 1) * D],
                                lhsT=xt_sb[:, c * XW + SH_PAD + t * 128:
                                           c * XW + SH_PAD + t * 128 + ncol],
                                rhs=w_sb[:, c * D:(c + 1) * D],
                                start=(c == 0),
                                stop=(c == KC - 1),
                            )
                    hp = phg.tile([128, PB * D], f16, tag="hp", name=f"hp{blo}")
                    nc.scalar.copy(out=hp[:, :(bhi - blo) * D],
                                   in_=ppad[:, :(bhi - blo) * D])
                    for cs in range(CS):
                        full = [t for t in range(blo, bhi) if PADB - t * 128 >= 128]
                        if full:
                            nc.gpsimd.dma_start(
                                out=bass.AP(h_padlocs[_rep % 2][cs],
                                            full[0] * 128 * DC,
                                            [[DC, 128], [128 * DC, len(full)],
                                             [1, DC]]),
                                in_=bass.AP(hg0 := hp.tensor,
                                            hp.offset + (full[0] - blo) * D + cs * DC,
                                            [[hp.tensor.shape[-1], 128],
                                             [D, len(full)], [1, DC]]),
                            )
                        for t in range(blo, bhi):
                            nr = PADB - t * 128
                            if nr >= 128:
                                continue
                            nc.gpsimd.dma_start(
                                out=bass.AP(h_padlocs[_rep % 2][cs], t * 128 * DC,
                                            [[DC, nr], [1, DC]]),
                                in_=bass.AP(hp.tensor,
                                            hp.offset + (t - blo) * D + cs * DC,
                                            [[hp.tensor.shape[-1], nr], [1, DC]]),
                            )
                if "B" in phases:
                    for cs in range(CS):
                        nc.gpsimd.collective_compute(
                            "AllGather",
                            mybir.AluOpType.bypass,
                            replica_groups=[list(range(M))],
                            ins=[h_padlocs[_rep % 2][cs].ap().opt()],
                            outs=[h_fullp[cs][N * DC:(N + SH_PAD) * DC].opt()],
                        )

            chunks = []
            if len(bounds) == 3:
                chunks.append((0, MAIN))
                chunks.append((MAIN, bounds[1]))
                chunks.append((bounds[1], NT))
            else:
                chunks.append((0, NT))
            copy_after = {hi - 1: (lo, hi) for (lo, hi) in chunks}

            pt = psum.tile([128, SGMAX * D], f32, tag="pt")
            copies, mms = {}, {}
            for t in range(NT):
                r = region_of(t)
                for c in range(KC):
                    mm = nc.tensor.matmul(
                        out=pt[:, r * D:(r + 1) * D],
                        lhsT=xt_sb[:, c * XW + t * 128: c * XW + (t + 1) * 128],
                        rhs=w_sb[:, c * D:(c + 1) * D],
                        start=(c == 0),
                        stop=(c == KC - 1),
                    )
                    mms[(t, c)] = mm
                if len(bounds) == 3 and t == bounds[1] - 2:
                    tile.add_dep_helper(
                        mms[(t, 0)].ins, copies[MAIN - 1].ins, sync=True,
                        reason="absorb main-copy ACT tick",
                    )
                if t in copy_after:
                    lo, hi = copy_after[t]
                    regs = sorted(region_of(u) for u in range(lo, hi))
                    rlo, rhi = regs[0], regs[-1] + 1
                    assert regs == list(range(rlo, rhi))
                    hg = phg.tile([128, SGMAX * D], f16, tag="hg", name=f"hg{lo}")
                    cp = nc.scalar.copy(
                        out=hg[:, :(rhi - rlo) * D],
                        in_=pt[:, rlo * D:rhi * D],
                    )
                    copies[t] = cp
                    strips = [u for u in range(lo, hi) if SH - u * 128 > 0]
                    full = [u for u in strips if SH - u * 128 >= 128]
                    partial = [u for u in strips if u not in full]
                    desc = region_of(lo) > region_of(lo + 1) if hi - lo > 1 else False
                    for cs in range(CS):
                        if full:
                            v = [region_of(u) - rlo for u in full]
                            if desc:
                                vmin = min(v)
                                u_at_vmin = full[v.index(vmin)]
                                out_ap = bass.AP(
                                    h_loc[cs], u_at_vmin * 128 * DC,
                                    [[DC, 128], [-128 * DC, len(full)], [1, DC]],
                                )
                            else:
                                out_ap = bass.AP(
                                    h_loc[cs], full[0] * 128 * DC,
                                    [[DC, 128], [128 * DC, len(full)], [1, DC]],
                                )
                                vmin = v[0]
                            nc.gpsimd.dma_start(
                                out=out_ap,
                                in_=bass.AP(
                                    hg.tensor, hg.offset + vmin * D + cs * DC,
                                    [[hg.tensor.shape[-1], 128],
                                     [D, len(full)], [1, DC]]),
                            )
                        for u in partial:
                            nr = SH - u * 128
                            ro = region_of(u) - rlo
                            nc.gpsimd.dma_start(
                                out=bass.AP(h_loc[cs], u * 128 * DC,
                                            [[DC, nr], [1, DC]]),
                                in_=bass.AP(
                                    hg.tensor, hg.offset + ro * D + cs * DC,
                                    [[hg.tensor.shape[-1], nr], [1, DC]]),
                            )

            # ---------------- AllGather + pad ----------------
            if "B" not in phases:
                continue
            for cs in range(CS):
                nc.gpsimd.collective_compute(
                    "AllGather",
                    mybir.AluOpType.bypass,
                    replica_groups=[list(range(M))],
                    ins=[h_loc[cs].ap().opt()],
                    outs=[h_fullp[cs][0:N * DC].opt()],
                )
            if not PAD2:
                pad_engs = [nc.sync, nc.scalar]
                for cs in range(CS):
                    pad_engs[cs % 2].dma_start(
                        out=h_fullp[cs][N * DC:(N + SH_PAD) * DC],
                        in_=h_fullp[cs][0:SH_PAD * DC])

            # ---------------- phase B: shifted-window accumulate ----------------
            if "W" not in phases:
                continue
            accs = [pacc.tile([128, FREEC], f16, tag=f"acc{c}", name=f"acc{c}")
                    for c in range(CS)]
            win_engs = [nc.sync, nc.scalar]
            for j in range(DEG):
                wts = []
                for cs in range(CS):
                    eng = win_engs[cs % 2] if CS > 1 else win_engs[j % 2]
                    wt = pwin.tile([128, FREEC], f16, tag=f"win{cs}",
                                   name=f"win{j}_{cs}")
                    with eng.register(f"st{_rep}_{j}_{cs}") as reg:
                        eng.reg_load(reg, starts[j:j + 1, 0:1])
                        sv = eng.snap(reg, min_val=0, max_val=N - 1)
                        eng.dma_start(
                            out=wt[:],
                            in_=bass.AP(h_fullp[cs], sv * DC,
                                        [[FREEC, 128], [1, FREEC]]),
                        )
                    wts.append(wt)
                if "n" in phases:
                    continue
                for cs in range(CS):
                    if j == 0:
                        nc.vector.tensor_copy(out=accs[cs][:], in_=wts[cs][:])
                    else:
                        nc.vector.tensor_add(
                            out=accs[cs][:], in0=accs[cs][:], in1=wts[cs][:])

            if "n" in phases:
                for cs in range(CS):
                    nc.vector.tensor_copy(out=accs[cs][:], in_=wts[cs][:])
            for cs in range(CS):
                win_engs[cs % 2].dma_start(
                    out=bass.AP(outs[cs], 0, [[FREEC, 128], [1, FREEC]]),
                    in_=accs[cs][:],
                )

    nc.compile()
    meta = dict(SH=SH, SH_PAD=SH_PAD, RPB=RPB, FREE=FREE, CS=CS, DC=DC)
    return nc, meta


def make_inputs(N, DIN, D, DEG, M, x, weight, bias, offsets, scale, PAD2=False):
    SH = N // M
    RPB = (SH + 127) // 128
    SH_PAD = 128 * RPB
    PADB = SH_PAD // M
    XW = SH_PAD + (PADB if PAD2 else 0)
    xt_full = np.ascontiguousarray(x.T).astype(np.float16)
    w_eff = (weight.astype(np.float32) * np.float32(scale)).astype(np.float16)
    in_maps = []
    for k in range(M):
        xt_k = np.zeros((DIN, XW), np.float16)
        xt_k[:, :SH] = xt_full[:, k * SH:(k + 1) * SH]
        if PAD2:
            # redundant pad slice: h rows [k*PADB, (k+1)*PADB) of the ring
            lo, hi = k * PADB, (k + 1) * PADB
            src = xt_full[:, lo:min(hi, N)]
            xt_k[:, SH_PAD:SH_PAD + src.shape[1]] = src
        starts_k = ((k * SH + offsets) % N).astype(np.int32)[:, None]
        in_maps.append({
            "xt": xt_k,
            "w": w_eff,
            "starts": starts_k,
        })
    return in_maps


_CACHE = {}
CS_DEFAULT = 2
PAD2_DEFAULT = False


def _get_nc(N, DIN, D, DEG, M):
    key = (N, DIN, D, DEG, M, CS_DEFAULT, PAD2_DEFAULT)
    if key not in _CACHE:
        _CACHE[key] = build_gcn_nc(N, DIN, D, DEG, M,
                                   CS=CS_DEFAULT, PAD2=PAD2_DEFAULT)
    return _CACHE[key]


def _is_circulant(N, DEG, rowptr, colind, colptr):
    if rowptr.shape[0] != N + 1 or colind.shape[0] != N * DEG:
        return None
    if not np.array_equal(rowptr.astype(np.int64),
                          np.arange(N + 1, dtype=np.int64) * DEG):
        return None
    if not np.array_equal(colptr, rowptr):
        return None
    offsets = colind[:DEG].astype(np.int64)
    if offsets.min() < 1 or offsets.max() >= N or len(set(offsets.tolist())) != DEG:
        return None
    rows = np.arange(N, dtype=np.int64)
    expect = np.sort((rows[:, None] + offsets[None, :]) % N, axis=1).reshape(-1)
    if not np.array_equal(colind.astype(np.int64), expect):
        return None
    return offsets


def _kernel_numpy_fallback(x, weight, bias, rowptr, colind, colptr):
    h = x.astype(np.float32) @ weight.astype(np.float32)
    out_deg = (colptr[1:] - colptr[:-1]).astype(np.float32)
    in_deg = (rowptr[1:] - rowptr[:-1]).astype(np.float32)
    h = h * (1.0 / np.sqrt(np.maximum(out_deg, 1e-30)))[:, None]
    N = rowptr.shape[0] - 1
    E = colind.shape[0]
    row_ids = np.searchsorted(rowptr, np.arange(E), side="right") - 1
    aggr = np.zeros_like(h)
    np.add.at(aggr, row_ids, h[colind])
    aggr = aggr * (1.0 / np.sqrt(np.maximum(in_deg, 1e-30)))[:, None]
    return (aggr + bias).astype(np.float32)


def kernel(x, weight, bias, rowptr, colind, colptr, rowind=None, **_unused):
    from concourse.bass_utils import run_bass_kernel_spmd

    x = np.asarray(x)
    weight = np.asarray(weight)
    bias = np.asarray(bias)
    rowptr = np.asarray(rowptr)
    colind = np.asarray(colind)
    colptr = np.asarray(colptr)

    N, DIN = x.shape
    D = weight.shape[1]
    M = 8
    DEG = colind.shape[0] // max(N, 1)

    offsets = _is_circulant(N, DEG, rowptr, colind, colptr)
    if offsets is None or N % M != 0 or DIN % 128 != 0:
        return _kernel_numpy_fallback(x, weight, bias, rowptr, colind, colptr)

    scale = 1.0 / DEG

    nc, meta = _get_nc(N, DIN, D, DEG, M)
    in_maps = make_inputs(N, DIN, D, DEG, M, x, weight, bias, offsets, scale,
                          PAD2=PAD2_DEFAULT)
    res = run_bass_kernel_spmd(nc, in_maps, list(range(M)))
    SH = meta["SH"]
    CS = meta["CS"]
    out = np.concatenate(
        [np.hstack([np.asarray(res.results[k][f"out{c}"])[:SH]
                    for c in range(CS)])
         for k in range(M)], axis=0
    )
    return (out.astype(np.float32) + bias.astype(np.float32)[None, :])
